# revision 24
# baseline (speedup 1.0000x reference)
"""Tensor-parallel GQA attention prefill block for 8 Trainium2 NeuronCores.

Problem (hardcoded): x:[2,1024,4096] f32, 32 Q heads / 8 KV heads, head dim
128, RoPE at positions arange(1024), causal mask, KV-cache positions >=1024
masked out (cache starts zeroed), output projection Wo. The computation
reduces exactly to causal GQA attention + o_proj.

Sharding: tensor-parallel over heads. Core c owns Q heads 4c..4c+3 and KV
head c (Wq/Wk/Wv column shards), computes attention for its heads over all
tokens, then per-head AllToAlls exchange attention outputs so each core
holds all 4096 features for a 128-token slice per batch; o_proj runs
token-sharded with the full (bf16) Wo; host concatenates the token slices.

v2 schedule: the PE is power-throttled to ~1.95GHz when busy, so the kernel
is PE-roofline-bound; the schedule keeps the PE stream dense end-to-end:
 - QKV runs as (grp, m, n) quarter-passes with 1-PSUM-bank accumulators so
   attention and QKV can share the 8 banks.
 - QKV(b1) matmul quanta are interleaved between attention(b0) steps (they
   cover the softmax ACT/DVE latency); o_proj(b0, dmq0) quanta likewise
   cover attention(b1) and the b1 AllToAll tail.
 - Scores matmuls are causally pruned (only key blocks <= query block), with
   a single [128,128] triangular multiplicative mask for diagonal blocks.
 - AllToAll is split per head so the collective pipeline starts 3 units
   before attention finishes.
 - Bulk weight/x DMAs are batched into multi-chunk descriptors and issued
   from two queues (SP + GPSIMD) to avoid issue serialization.
"""
import sys

sys.path.insert(0, "/opt/trn_rl_repo")

import numpy as np
import ml_dtypes

import concourse.bass as bass
import concourse.tile as tile
from concourse import mybir
from concourse.bass import ts
from concourse.bass_utils import run_bass_kernel_spmd

BF16 = mybir.dt.bfloat16
F32 = mybir.dt.float32
AF = mybir.ActivationFunctionType
OP = mybir.AluOpType

B, S, D = 2, 1024, 4096
H, KVH, HD = 32, 8, 128
NC = 8
QH = H // NC  # 4 q heads per core
THETA = 1000000.0
SC = 1.0 / float(np.sqrt(HD))

RG = [list(range(NC))]


def _build(split_for_walrus=True):
    nc = bass.Bass("TRN2", num_devices=NC)

    xT = nc.declare_dram_parameter("xT", [B, D, S], BF16, isOutput=False)
    wA = nc.declare_dram_parameter("wA", [D, 256], BF16, isOutput=False)
    wB = nc.declare_dram_parameter("wB", [D, 256], BF16, isOutput=False)
    wC = nc.declare_dram_parameter("wC", [D, 256], BF16, isOutput=False)
    wo = nc.declare_dram_parameter("wo", [D, D], BF16, isOutput=False)
    bias6 = nc.declare_dram_parameter("bias6", [6, 128], F32, isOutput=False)
    cosT = nc.declare_dram_parameter("cosT", [128, S], BF16, isOutput=False)
    sinT = nc.declare_dram_parameter("sinT", [128, S], BF16, isOutput=False)
    tri = nc.declare_dram_parameter("tri", [128, 128], BF16, isOutput=False)
    ident = nc.declare_dram_parameter("ident", [128, 128], BF16, isOutput=False)
    out = nc.declare_dram_parameter("out", [B, 128, D], F32, isOutput=True)

    from contextlib import ExitStack

    with ExitStack() as es:
        tc = es.enter_context(tile.TileContext(nc))
        cpool = es.enter_context(tc.tile_pool(name="consts", bufs=1))
        xcpool = es.enter_context(tc.tile_pool(name="xc", bufs=16))
        wpool = es.enter_context(tc.tile_pool(name="wslab", bufs=8))
        wopool = es.enter_context(tc.tile_pool(name="wo", bufs=4))
        ropepool = es.enter_context(tc.tile_pool(name="rope", bufs=2))
        kqvpool = es.enter_context(tc.tile_pool(name="kqv", bufs=12))
        ppool = es.enter_context(tc.tile_pool(name="attn", bufs=12))
        ptsbpool = es.enter_context(tc.tile_pool(name="ptsb", bufs=10))
        sumpool = es.enter_context(tc.tile_pool(name="sums", bufs=8))
        dgpool = es.enter_context(tc.tile_pool(name="diag", bufs=12))
        atpool = es.enter_context(tc.tile_pool(name="at", bufs=5))
        gpool = es.enter_context(tc.tile_pool(name="g", bufs=2))
        ypool = es.enter_context(tc.tile_pool(name="ysb", bufs=3))
        pssp = es.enter_context(tc.tile_pool(name="pssp", bufs=3, space="PSUM"))
        pspt = es.enter_context(tc.tile_pool(name="pspt", bufs=2, space="PSUM"))
        psot = es.enter_context(tc.tile_pool(name="psot", bufs=1, space="PSUM"))
        psqa = es.enter_context(tc.tile_pool(name="psqa", bufs=2, space="PSUM"))
        dpool = es.enter_context(tc.tile_pool(name="dram", bufs=8, space="DRAM"))

        # ---------------- constants ----------------
        cos_sb = cpool.tile([128, S], BF16, tag="cos", name="cos")
        sin_sb = cpool.tile([128, S], BF16, tag="sin", name="sin")
        tri_sb = cpool.tile([128, 128], BF16, tag="tri", name="tri")
        id_sb = cpool.tile([128, 128], BF16, tag="ident", name="ident")
        b_sb = cpool.tile([128, 6], F32, tag="bias", name="bias")
        # consts go via the vector queue: the SP queue must start issuing
        # x-chunk DMAs immediately (the PE waits on those)
        nc.scalar.dma_start(cos_sb[:], cosT[:])
        nc.scalar.dma_start(sin_sb[:], sinT[:])
        nc.scalar.dma_start(tri_sb[:], tri[:])
        nc.scalar.dma_start(id_sb[:], ident[:])
        nc.scalar.dma_start(b_sb[:], bias6[:].rearrange("i p -> p i"))

        # per-batch state
        rot = [{}, {}]   # b -> mg -> rotated [128, S] bf16 (mg: 0=Q0 1=K 3=Q1 4=Q2 5=Q3)
        vsb = [None, None]
        at = [[None] * QH, [None] * QH]
        G = [None, None]

        # ------------- QKV machinery (quantized into closures) -------------
        GRPS = ((0, wA), (1, wB), (2, wC))

        def qkv_quanta(b):
            """Return a list of closures; each emits ~1us of PE work (4 MMs)
            plus its own DMA/ACT/DVE trailing ops. All tile allocation happens
            at closure RUN time so pool WAR deps see already-emitted readers.
            xc DMAs on SP queue, slab DMAs on GPSIMD queue (parallel issue)."""
            Q = []
            xc = [[None] * 8 for _ in range(2)]  # [n][kq] -> [128, 2048] bf16
            slabsets = [[None] * 8 for _ in range(3)]
            state = {"v_sb": [], "ps": None}

            def load_xc(kq, n):
                t = xcpool.tile([128, 2048], BF16, tag="xc", name="xc")
                nc.sync.dma_start(
                    t[:].rearrange("p (c s) -> p c s", c=4),
                    xT[b, ts(kq, 512), ts(n, 512)].rearrange(
                        "(c p) s -> p c s", c=4
                    ),
                )
                xc[n][kq] = t

            def load_slab(gi, s):
                t = wpool.tile([128, 1024], BF16, tag="wslab", name="wslab")
                nc.gpsimd.dma_start(
                    t[:].rearrange("p (c m) -> p c m", c=4),
                    GRPS[gi][1][ts(s, 512), :].rearrange(
                        "(c p) m -> p c m", c=4
                    ),
                )
                slabsets[gi][s] = t

            def prefetch():
                for kq in range(2):
                    load_xc(kq, 0)
                    load_xc(kq, 1)
                load_slab(0, 0)
                load_slab(0, 1)
            Q.append(prefetch)

            def epilogue(mg, n, b=b):
                ps = state["ps"]
                if mg != 2:
                    if n == 0:
                        rot[b][mg] = kqvpool.tile(
                            [128, S], BF16, tag="kqv", name="rot"
                        )
                    q32 = ropepool.tile([128, 512], F32, tag="q32", name="q32")
                    nc.scalar.activation(
                        q32[:], ps[:], AF.Identity, bias=b_sb[:, mg:mg + 1]
                    )
                    sh = ropepool.tile([128, 512], F32, tag="sh", name="sh")
                    nc.gpsimd.dma_start(sh[0:64, :], q32[64:128, :])
                    nc.gpsimd.dma_start(sh[64:128, :], q32[0:64, :])
                    nc.vector.tensor_mul(q32[:], q32[:], cos_sb[:, ts(n, 512)])
                    nc.vector.tensor_mul(sh[:], sh[:], sin_sb[:, ts(n, 512)])
                    nc.vector.tensor_add(
                        rot[b][mg][:, ts(n, 512)], q32[:], sh[:]
                    )
                else:
                    vt = ropepool.tile([128, 512], BF16, tag="vt", name="vt")
                    nc.scalar.activation(
                        vt[:], ps[:], AF.Identity, bias=b_sb[:, 2:3]
                    )
                    state["v_sb"].append(vt)

            # single sweep over x per grp: 4 psum quarters (m,n) accumulate
            # together, so each xc tile is consumed once per grp at ~4.2us
            # per 1MB — well under HBM delivery rate (no DMA-paced stalls).
            # Quarters borrow the idle attention "sp" psum ring (2 qa + 2 sp).
            for gi in range(3):
                for kg in range(8):  # 8 quanta of 16 MMs each
                    def quantum(kg=kg, gi=gi):
                        if kg == 0:
                            state["qs"] = {
                                (0, 0): psqa.tile(
                                    [128, 512], F32, tag="qa", name="qkvps"
                                ),
                                (0, 1): psqa.tile(
                                    [128, 512], F32, tag="qa", name="qkvps"
                                ),
                                (1, 0): pssp.tile(
                                    [128, 512], F32, tag="sp", name="qkvps"
                                ),
                                (1, 1): pssp.tile(
                                    [128, 512], F32, tag="sp", name="qkvps"
                                ),
                            }
                        qs = state["qs"]
                        # quarter-grouped MM order: quarter i's first MM lands
                        # ~1.05*i us into the quantum, past the point where the
                        # previous grp's epilogue act i has freed the ring slot
                        for m in range(2):
                            for n in range(2):
                                for c4 in range(4):
                                    k = kg * 4 + c4
                                    kq, cc = k // 4, k % 4
                                    nc.tensor.matmul(
                                        qs[(m, n)][:],
                                        slabsets[gi][kq][
                                            :, cc * 256 + m * 128:
                                            cc * 256 + m * 128 + 128
                                        ],
                                        xc[n][kq][:, ts(cc, 512)],
                                        start=(k == 0),
                                        stop=(k == 31),
                                    )
                        # staging (after the MMs so WAR deps resolve fast)
                        if kg < 6:
                            load_slab(gi, kg + 2)
                        elif gi < 2:
                            load_slab(gi + 1, kg - 6)
                        if gi == 0 and kg < 6:
                            load_xc(kg + 2, 0)
                            load_xc(kg + 2, 1)
                        if kg == 7:
                            for m in range(2):
                                mg = gi * 2 + m
                                for n in range(2):
                                    state["ps"] = qs[(m, n)]
                                    epilogue(mg, n)
                    Q.append(quantum)

                if gi == 1:
                    # V transpose right after grp B (V = grp B, m=0)
                    def vtrans(half, b=b):
                        if half == 0:
                            vsb[b] = kqvpool.tile(
                                [128, S], BF16, tag="kqv", name="vsb"
                            )
                        for j in range(4):
                            jj = half * 4 + j
                            vt = state["v_sb"][jj // 4]
                            vp = psqa.tile([128, 128], F32, tag="qa", name="vp")
                            nc.tensor.matmul(
                                vp[:], vt[:, ts(jj % 4, 128)], id_sb[:],
                                start=True, stop=True,
                            )
                            nc.vector.tensor_copy(
                                vsb[b][:, ts(jj, 128)], vp[:]
                            )
                    Q.append(lambda: vtrans(0))
                    Q.append(lambda: vtrans(1))
            return Q

        def run_qkv_inline(b):
            for q in qkv_quanta(b):
                q()

        # ------------- attention machinery -------------
        def emit_scores_softmax(b, h, g, pump):
            """Causal mask is folded into the PE stream: the diagonal 128-
            block gets an extra accumulating matmul id^T @ trineg (additive
            -1e5 above the diagonal), so softmax is just exp(+accum_out),
            reciprocal, and the diag-scale tile — a 3-hop chain the depth-2
            pipeline fully hides."""
            Q_t = rot[b][(0, 3, 4, 5)[h]]
            K_t = rot[b][1]
            plist = []
            for j in range(4):
                W = g * 512 + (j + 1) * 128
                qi = 4 * g + j
                P = ppool.tile([128, W], BF16, tag="psb", name="psb")
                sums = sumpool.tile([128, 1], F32, tag="sums", name="sums")
                if W > 512:
                    spA = pssp.tile([128, 512], F32, tag="sp", name="spA")
                    nc.tensor.matmul(
                        spA[:], Q_t[:, ts(qi, 128)], K_t[:, 0:512],
                        start=True, stop=True,
                    )
                    pump(1)
                    WB = W - 512
                    spB = pssp.tile([128, WB], F32, tag="sp", name="spB")
                    nc.tensor.matmul(
                        spB[:], Q_t[:, ts(qi, 128)], K_t[:, 512:W],
                        start=True, stop=False,
                    )
                    nc.tensor.matmul(
                        spB[:, WB - 128:WB], id_sb[:], tri_sb[:],
                        start=False, stop=True,
                    )
                    nc.scalar.activation(
                        P[:, 0:512], spA[:], AF.Exp, scale=SC,
                        accum_out=sums[:],
                    )
                    pump(1)
                    sums2 = sumpool.tile([128, 1], F32, tag="sums2", name="sums2")
                    nc.scalar.activation(
                        P[:, 512:W], spB[:], AF.Exp, scale=SC,
                        accum_out=sums2[:],
                    )
                    nc.vector.tensor_add(sums[:], sums[:], sums2[:])
                else:
                    sp = pssp.tile([128, W], F32, tag="sp", name="sp")
                    nc.tensor.matmul(
                        sp[:], Q_t[:, ts(qi, 128)], K_t[:, 0:W],
                        start=True, stop=False,
                    )
                    nc.tensor.matmul(
                        sp[:, W - 128:W], id_sb[:], tri_sb[:],
                        start=False, stop=True,
                    )
                    pump(1)
                    nc.scalar.activation(
                        P[:], sp[:], AF.Exp, scale=SC, accum_out=sums[:]
                    )
                recip = sumpool.tile([128, 1], F32, tag="recip", name="recip")
                nc.vector.reciprocal(recip[:], sums[:])
                Dt = dgpool.tile([128, 128], BF16, tag="diag", name="diag")
                nc.vector.tensor_scalar_mul(Dt[:], id_sb[:], recip[:])
                plist.append((P, Dt))
                pump(1)
            return plist

        def emit_pt(g, plist, pump):
            pts = []
            for kc in range(4 * g + 4):
                jst = max(0, kc - 4 * g)
                ptp = pspt.tile([128, 512], F32, tag="ptp", name="ptp")
                for j in range(jst, 4):
                    nc.tensor.matmul(
                        ptp[:, ts(j, 128)],
                        plist[j][0][:, ts(kc, 128)],
                        plist[j][1][:],
                        start=True, stop=True,
                    )
                pt = ptsbpool.tile([128, 512], BF16, tag="ptsb", name="ptsb")
                if kc % 2 == 0:
                    nc.vector.tensor_copy(
                        pt[:, jst * 128:512], ptp[:, jst * 128:512]
                    )
                else:
                    nc.scalar.copy(
                        pt[:, jst * 128:512], ptp[:, jst * 128:512]
                    )
                pts.append((pt, jst))
                if kc % 2 == 1:
                    pump(1)
            return pts

        def emit_ot(b, h, g, pts):
            ot = psot.tile([128, 512], F32, tag="ot", name="ot")
            nkc = 4 * g + 4
            for kc in range(nkc):
                pt, jst = pts[kc]
                nc.tensor.matmul(
                    ot[:, jst * 128:512],
                    vsb[b][:, ts(kc, 128)],
                    pt[:, jst * 128:512],
                    start=(kc == 0), stop=(kc == nkc - 1),
                )
            if at[b][h] is None:
                at[b][h] = atpool.tile([128, S], BF16, tag="at", name="at")
            nc.scalar.copy(at[b][h][:, ts(g, 512)], ot[:])

        a2o_pending = {}  # key -> a2a output dram tile

        def emit_a2a_trigger_full(b):
            """Single AllToAll for all 4 heads of batch b (1MB)."""
            a2i = dpool.tile([NC, 512, 128], BF16, tag="a2iF", name="a2iF")
            for h in range(QH):
                nc.gpsimd.dma_start(
                    a2i[:, h * 128:(h + 1) * 128, :].rearrange(
                        "d p t -> p d t"
                    ),
                    at[b][h][:].rearrange("p (d t) -> p d t", d=NC),
                )
            a2o = dpool.tile([NC, 512, 128], BF16, tag="a2oF", name="a2oF")
            nc.gpsimd.collective_compute(
                "AllToAll",
                OP.bypass,
                ins=[a2i[:].opt()],
                outs=[a2o[:].opt()],
                replica_groups=RG,
            )
            a2o_pending[(b, "full")] = a2o

        def emit_a2a_gather_full(b):
            if G[b] is None:
                G[b] = gpool.tile([128, 4096], BF16, tag="g", name="g")
            a2o = a2o_pending.pop((b, "full"))
            nc.gpsimd.dma_start(
                G[b][:].rearrange("p (fc t) -> p fc t", fc=32),
                a2o[:].rearrange("s (fl p) t -> p (s fl) t", p=128),
            )

        def emit_a2a_trigger(b, hbase):
            """AllToAll for heads hbase, hbase+1 of batch b (512KB)."""
            a2i = dpool.tile([NC, 256, 128], BF16, tag="a2i", name="a2i")
            for hl in range(2):
                nc.gpsimd.dma_start(
                    a2i[:, hl * 128:(hl + 1) * 128, :].rearrange(
                        "d p t -> p d t"
                    ),
                    at[b][hbase + hl][:].rearrange("p (d t) -> p d t", d=NC),
                )
            a2o = dpool.tile([NC, 256, 128], BF16, tag="a2o", name="a2o")
            nc.gpsimd.collective_compute(
                "AllToAll",
                OP.bypass,
                ins=[a2i[:].opt()],
                outs=[a2o[:].opt()],
                replica_groups=RG,
            )
            a2o_pending[(b, hbase)] = a2o

        def emit_a2a_gather(b, hbase):
            if G[b] is None:
                G[b] = gpool.tile([128, 4096], BF16, tag="g", name="g")
            a2o = a2o_pending.pop((b, hbase))
            for hl in range(2):
                nc.gpsimd.dma_start(
                    G[b][:].rearrange(
                        "p (s four t) -> p s four t", s=NC, four=QH
                    )[:, :, hbase + hl, :],
                    a2o[:, hl * 128:(hl + 1) * 128, :].rearrange(
                        "s p t -> p s t"
                    ),
                )

        def emit_warmup_collective():
            """Tiny AllToAll to absorb the ~11us first-collective spin-up
            while the PE is busy with QKV(b0)."""
            wi = dpool.tile([NC, 1, 128], BF16, tag="wi", name="wi")
            nc.gpsimd.dma_start(
                wi[:].rearrange("d o t -> o d t")[0],
                cos_sb[0:1, 0:NC * 128].rearrange("o (d t) -> o d t", d=NC)[0],
            )
            wo_ = dpool.tile([NC, 1, 128], BF16, tag="wu", name="wu")
            nc.gpsimd.collective_compute(
                "AllToAll",
                OP.bypass,
                ins=[wi[:].opt()],
                outs=[wo_[:].opt()],
                replica_groups=RG,
            )

        def run_attention_interleaved(filler, fill_from=13):
            """Both batches' attention, unit-interleaved 2:1 (b0-heavy early)
            so b0's AllToAll fires at ~62% of the phase. Fillers (o_proj b0
            quanta) are pumped only from unit `fill_from` on — after b0's
            gather has landed."""
            fill = {"q": list(filler), "i": 0}

            def pump(n):
                if fill["i"] < fill_from:
                    return
                for _ in range(n):
                    if fill["q"]:
                        f = fill["q"].pop(0)
                        if f is not None:
                            f()

            order = [
                (0, 0, 0), (0, 0, 1), (1, 0, 0),
                (0, 1, 0), (0, 1, 1), (1, 0, 1),
                (0, 2, 0), (0, 2, 1), (1, 1, 0),
                (0, 3, 0), (0, 3, 1), (1, 1, 1),
                (1, 2, 0), (1, 2, 1), (1, 3, 0), (1, 3, 1),
            ]  # (b, h, g)
            plists = {
                0: emit_scores_softmax(*order[0], pump),
                1: emit_scores_softmax(*order[1], pump),
            }
            for i in range(len(order)):
                fill["i"] = i
                if i + 2 < len(order):
                    plists[i + 2] = emit_scores_softmax(*order[i + 2], pump)
                b, h, g = order[i]
                pts = emit_pt(g, plists.pop(i), pump)
                emit_ot(b, h, g, pts)
                pump(1)
                if i == 10:
                    # all of b0's heads done: single 1MB AllToAll + gather
                    emit_a2a_trigger_full(0)
                    emit_a2a_gather_full(0)
                elif i == 11:
                    emit_a2a_trigger(1, 0)
                    emit_a2a_gather(1, 0)
                elif i == 15:
                    emit_a2a_trigger(1, 2)
                    emit_a2a_gather(1, 2)
            return fill["q"]  # leftovers

        # ------------- o_proj machinery -------------
        def load_wo_pair(fcp, dmq):
            t = wopool.tile([128, 2048], BF16, tag="wo", name="wopair")
            nc.sync.dma_start(
                t[:].rearrange("p (c q) -> p c q", c=2),
                wo[ts(fcp, 256), ts(dmq, 1024)].rearrange(
                    "(c p) q -> p c q", c=2
                ),
            )
            return t

        def oproj_single_quanta(b, dmq, split_heads=False):
            """o_proj for one batch, one dmq chunk: 2 psum quarters held
            across 16 quanta. With split_heads, even fcp pairs (head-local
            features 0-1 of every core, available after the h01 AllToAll)
            run before odd pairs (h23)."""
            Q = []
            yps = {}
            pairs = {}
            fcp_order = (
                list(range(0, 16, 2)) + list(range(1, 16, 2))
                if split_heads else list(range(16))
            )

            def start():
                yps[0] = psqa.tile([128, 512], F32, tag="qa", name="yp0")
                yps[1] = psqa.tile([128, 512], F32, tag="qa", name="yp1")
                pairs[fcp_order[0]] = load_wo_pair(fcp_order[0], dmq)
                pairs[fcp_order[1]] = load_wo_pair(fcp_order[1], dmq)
            Q.append(start)

            for idx in range(16):
                def quantum(idx=idx):
                    if idx + 2 < 16:
                        nxt = fcp_order[idx + 2]
                        pairs[nxt] = load_wo_pair(nxt, dmq)
                    fcp = fcp_order[idx]
                    pair = pairs.pop(fcp)
                    for c in range(2):
                        fc = 2 * fcp + c
                        for n in range(2):
                            nc.tensor.matmul(
                                yps[n][:],
                                G[b][:, ts(fc, 128)],
                                pair[:, c * 1024 + n * 512:
                                     c * 1024 + (n + 1) * 512],
                                start=(idx == 0 and c == 0),
                                stop=(idx == 15 and c == 1),
                            )
                Q.append(quantum)

            def finish():
                for n in range(2):
                    ys = ypool.tile([128, 512], F32, tag="ysb", name="ys")
                    if n == 0:
                        nc.scalar.copy(ys[:], yps[n][:])
                    else:
                        nc.vector.tensor_copy(ys[:], yps[n][:])
                    nc.gpsimd.dma_start(
                        out[b, :, dmq * 1024 + n * 512:
                            dmq * 1024 + (n + 1) * 512],
                        ys[:],
                    )
            Q.append(finish)
            return Q

        def oproj_joint(dmq):
            """o_proj for both batches on one dmq chunk, fcp-interleaved so
            each wo pair tile is consumed immediately. 4 psum quarters:
            2 from qa tag, 2 from sp tag (attention is done by now)."""
            yps = {}
            yps[(0, 0)] = psqa.tile([128, 512], F32, tag="qa", name="yp00")
            yps[(0, 1)] = psqa.tile([128, 512], F32, tag="qa", name="yp01")
            yps[(1, 0)] = pssp.tile([128, 512], F32, tag="sp", name="yp10")
            yps[(1, 1)] = pssp.tile([128, 512], F32, tag="sp", name="yp11")
            pairs = {0: load_wo_pair(0, dmq), 1: load_wo_pair(1, dmq)}
            for fcp in range(16):
                if fcp + 2 < 16:
                    pairs[fcp + 2] = load_wo_pair(fcp + 2, dmq)
                pair = pairs.pop(fcp)
                for b in range(B):
                    for c in range(2):
                        fc = 2 * fcp + c
                        for n in range(2):
                            nc.tensor.matmul(
                                yps[(b, n)][:],
                                G[b][:, ts(fc, 128)],
                                pair[:, c * 1024 + n * 512:
                                     c * 1024 + (n + 1) * 512],
                                start=(fc == 0), stop=(fc == 31),
                            )
            # drain: alternate copy engines so the 4 copies pipeline
            for b in range(B):
                for n in range(2):
                    ys = ypool.tile([128, 512], F32, tag="ysb", name="ys")
                    if (b + n) % 2 == 0:
                        nc.scalar.copy(ys[:], yps[(b, n)][:])
                    else:
                        nc.vector.tensor_copy(ys[:], yps[(b, n)][:])
                    nc.gpsimd.dma_start(
                        out[b, :, dmq * 1024 + n * 512:
                            dmq * 1024 + (n + 1) * 512],
                        ys[:],
                    )

        # ================= schedule =================
        # Phase 1: QKV(b0) inline (dense PE stream, DMA-paced ramp)
        run_qkv_inline(0)
        # absorb the ~11us first-collective spin-up under QKV(b1)
        emit_warmup_collective()
        # Phase 2: QKV(b1) inline
        run_qkv_inline(1)
        # Phase 3: attention, 2:1 unit-interleaved; no fillers (the PE queue
        # is in-order, so any filler waiting on a collective would block the
        # ready attention work emitted behind it)
        run_attention_interleaved([], fill_from=99)
        # o_proj(b0, dmq0): b0's collective completes right at phase-3 end
        for q in oproj_single_quanta(0, 0, split_heads=True):
            q()
        # Phase 4: o_proj(b1, dmq0), h01 pairs first (their AllToAll lands
        # before the h23 one); wo dmq0 re-streamed
        for q in oproj_single_quanta(1, 0, split_heads=True):
            q()
        # Phase 5: joint o_proj dmq 1..3
        for dmq in range(1, 4):
            oproj_joint(dmq)

    if split_for_walrus:
        _split_waits(nc, cap=1)
    return nc


def _split_waits(nc, cap=1):
    """This walrus build accepts at most one sync wait per instruction; hoist
    the excess onto same-engine NoOps inserted immediately before."""
    for fn in nc.m.functions:
        for bb in fn.blocks:
            new_insts = []
            for inst in bb.instructions:
                si = inst.sync_info
                if si is not None and si.on_wait and len(si.on_wait) > cap:
                    waits = list(si.on_wait)
                    head, rest = waits[: len(waits) - cap], waits[len(waits) - cap:]
                    for i in range(0, len(head), cap):
                        nop = mybir.InstNoOp(
                            name=f"{inst.name}-wsplit{i}", ins=[], outs=[]
                        )
                        nop.engine = inst.engine
                        nop.sync_info = mybir.SyncInfo(
                            on_wait=head[i : i + cap], on_update=[]
                        )
                        new_insts.append(nop)
                    inst.sync_info = mybir.SyncInfo(
                        on_wait=rest, on_update=list(si.on_update)
                    )
                new_insts.append(inst)
            bb.instructions = new_insts
    return nc


_NC_CACHE = None


def _get_nc():
    global _NC_CACHE
    if _NC_CACHE is None:
        _NC_CACHE = _build()
    return _NC_CACHE


def _prep_inputs(x, storage_idx, Wq, bq, Wk, bk, Wv, bv, Wo):
    bf = ml_dtypes.bfloat16
    xT = np.ascontiguousarray(
        np.asarray(x, np.float32).transpose(0, 2, 1)
    ).astype(bf)  # [B, D, S]
    wo_bf = np.ascontiguousarray(np.asarray(Wo, np.float32)).astype(bf)

    pos = np.asarray(storage_idx, np.int64).astype(np.float32)  # [S]
    inv = (1.0 / (THETA ** (np.arange(0, HD, 2, dtype=np.float32) / HD))).astype(
        np.float32
    )
    fr = pos[:, None] * inv[None, :]  # [S, 64]
    emb = np.concatenate([fr, fr], axis=1)  # [S, HD]
    cosT = np.ascontiguousarray(np.cos(emb).T.astype(np.float32)).astype(bf)
    sinT32 = np.ascontiguousarray(np.sin(emb).T).astype(np.float32)
    sinT32[0:64] *= -1.0
    sinT = sinT32.astype(bf)  # fold rotate_half sign

    # additive causal mask for diagonal 128-blocks (0 allowed, -1e5 masked);
    # accumulated onto the scores in-PE via id^T @ tri
    r = np.arange(128)[:, None]
    c = np.arange(128)[None, :]
    tri = np.where(c <= r, 0.0, -1e5).astype(np.float32).astype(bf)
    identity = np.eye(128, dtype=np.float32).astype(bf)

    in_maps = []
    for core in range(NC):
        q0 = core * 512
        kv = slice(core * 128, (core + 1) * 128)
        wA = np.ascontiguousarray(
            np.concatenate([Wq[:, q0 : q0 + 128], Wk[:, kv]], axis=1)
        ).astype(bf)
        wB = np.ascontiguousarray(
            np.concatenate([Wv[:, kv], Wq[:, q0 + 128 : q0 + 256]], axis=1)
        ).astype(bf)
        wC = np.ascontiguousarray(Wq[:, q0 + 256 : q0 + 512]).astype(bf)
        bias6 = np.stack(
            [
                np.asarray(bq[q0 : q0 + 128], np.float32),
                np.asarray(bk[core * 128 : (core + 1) * 128], np.float32),
                np.asarray(bv[core * 128 : (core + 1) * 128], np.float32),
                np.asarray(bq[q0 + 128 : q0 + 256], np.float32),
                np.asarray(bq[q0 + 256 : q0 + 384], np.float32),
                np.asarray(bq[q0 + 384 : q0 + 512], np.float32),
            ]
        )  # [6, 128]
        in_maps.append(
            {
                "xT": xT,
                "wA": wA,
                "wB": wB,
                "wC": wC,
                "wo": wo_bf,
                "bias6": np.ascontiguousarray(bias6),
                "cosT": cosT,
                "sinT": sinT,
                "tri": tri,
                "ident": identity,
            }
        )
    return in_maps


_LAST_RESULTS = None


def kernel(x, storage_idx, cache, mask, Wq, bq, Wk, bk, Wv, bv, Wo):
    """Full-input, full-output entry point. cache/mask are consumed implicitly:
    cache is zeros and positions >= S are causally masked, so the computation
    reduces to causal attention over the S prefill tokens."""
    global _LAST_RESULTS
    in_maps = _prep_inputs(x, storage_idx, Wq, bq, Wk, bk, Wv, bv, Wo)
    nc = _get_nc()
    res = run_bass_kernel_spmd(nc, in_maps, core_ids=list(range(NC)))
    _LAST_RESULTS = res
    full = np.empty((B, S, D), np.float32)
    for c in range(NC):
        o = res.results[c]["out"]  # [B, 128, D]
        for b in range(B):
            full[b, 128 * c : 128 * (c + 1), :] = o[b]
    return full


# revision 32
# speedup vs baseline: 1.0886x; 1.0886x over previous
"""Tensor-parallel GQA attention prefill block for 8 Trainium2 NeuronCores.

Problem (hardcoded): x:[2,1024,4096] f32, 32 Q heads / 8 KV heads, head dim
128, RoPE at positions arange(1024), causal mask, KV-cache positions >=1024
masked out (cache starts zeroed), output projection Wo. The computation
reduces exactly to causal GQA attention + o_proj.

Sharding: tensor-parallel over heads. Core c owns Q heads 4c..4c+3 and KV
head c (Wq/Wk/Wv column shards), computes attention for its heads over all
tokens, then per-head AllToAlls exchange attention outputs so each core
holds all 4096 features for a 128-token slice per batch; o_proj runs
token-sharded with the full (bf16) Wo; host concatenates the token slices.

v2 schedule: the PE is power-throttled to ~1.95GHz when busy, so the kernel
is PE-roofline-bound; the schedule keeps the PE stream dense end-to-end:
 - QKV runs as (grp, m, n) quarter-passes with 1-PSUM-bank accumulators so
   attention and QKV can share the 8 banks.
 - QKV(b1) matmul quanta are interleaved between attention(b0) steps (they
   cover the softmax ACT/DVE latency); o_proj(b0, dmq0) quanta likewise
   cover attention(b1) and the b1 AllToAll tail.
 - Scores matmuls are causally pruned (only key blocks <= query block), with
   a single [128,128] triangular multiplicative mask for diagonal blocks.
 - AllToAll is split per head so the collective pipeline starts 3 units
   before attention finishes.
 - Bulk weight/x DMAs are batched into multi-chunk descriptors and issued
   from two queues (SP + GPSIMD) to avoid issue serialization.
"""
import sys

sys.path.insert(0, "/opt/trn_rl_repo")

import numpy as np
import ml_dtypes

import concourse.bass as bass
import concourse.tile as tile
from concourse import mybir
from concourse.bass import ts
from concourse.bass_utils import run_bass_kernel_spmd

BF16 = mybir.dt.bfloat16
F32 = mybir.dt.float32
AF = mybir.ActivationFunctionType
OP = mybir.AluOpType

B, S, D = 2, 1024, 4096
H, KVH, HD = 32, 8, 128
NC = 8
QH = H // NC  # 4 q heads per core
THETA = 1000000.0
SC = 1.0 / float(np.sqrt(HD))

RG = [list(range(NC))]


def _build(split_for_walrus=True):
    nc = bass.Bass("TRN2", num_devices=NC)

    xT = nc.declare_dram_parameter("xT", [B, D, S], BF16, isOutput=False)
    wA = nc.declare_dram_parameter("wA", [D, 256], BF16, isOutput=False)
    wB = nc.declare_dram_parameter("wB", [D, 256], BF16, isOutput=False)
    wC = nc.declare_dram_parameter("wC", [D, 256], BF16, isOutput=False)
    wo = nc.declare_dram_parameter("wo", [D, D], BF16, isOutput=False)
    bias6 = nc.declare_dram_parameter("bias6", [6, 128], F32, isOutput=False)
    cosT = nc.declare_dram_parameter("cosT", [128, S], BF16, isOutput=False)
    sinT = nc.declare_dram_parameter("sinT", [128, S], BF16, isOutput=False)
    tri = nc.declare_dram_parameter("tri", [128, 128], BF16, isOutput=False)
    ident = nc.declare_dram_parameter("ident", [128, 128], BF16, isOutput=False)
    out = nc.declare_dram_parameter("out", [B, 128, D], F32, isOutput=True)

    from contextlib import ExitStack

    with ExitStack() as es:
        tc = es.enter_context(tile.TileContext(nc))
        cpool = es.enter_context(tc.tile_pool(name="consts", bufs=1))
        xcpool = es.enter_context(tc.tile_pool(name="xc", bufs=16))
        wpool = es.enter_context(tc.tile_pool(name="wslab", bufs=8))
        wopool = es.enter_context(tc.tile_pool(name="wo", bufs=4))
        ropepool = es.enter_context(tc.tile_pool(name="rope", bufs=2))
        kqvpool = es.enter_context(tc.tile_pool(name="kqv", bufs=12))
        ppool = es.enter_context(tc.tile_pool(name="attn", bufs=12))
        ptsbpool = es.enter_context(tc.tile_pool(name="ptsb", bufs=10))
        sumpool = es.enter_context(tc.tile_pool(name="sums", bufs=8))
        dgpool = es.enter_context(tc.tile_pool(name="diag", bufs=12))
        atpool = es.enter_context(tc.tile_pool(name="at", bufs=5))
        gpool = es.enter_context(tc.tile_pool(name="g", bufs=2))
        ypool = es.enter_context(tc.tile_pool(name="ysb", bufs=3))
        pssp = es.enter_context(tc.tile_pool(name="pssp", bufs=3, space="PSUM"))
        pspt = es.enter_context(tc.tile_pool(name="pspt", bufs=2, space="PSUM"))
        psot = es.enter_context(tc.tile_pool(name="psot", bufs=1, space="PSUM"))
        psqa = es.enter_context(tc.tile_pool(name="psqa", bufs=2, space="PSUM"))
        dpool = es.enter_context(tc.tile_pool(name="dram", bufs=8, space="DRAM"))

        # ---------------- constants ----------------
        cos_sb = cpool.tile([128, S], BF16, tag="cos", name="cos")
        sin_sb = cpool.tile([128, S], BF16, tag="sin", name="sin")
        tri_sb = cpool.tile([128, 128], BF16, tag="tri", name="tri")
        id_sb = cpool.tile([128, 128], BF16, tag="ident", name="ident")
        b_sb = cpool.tile([128, 6], F32, tag="bias", name="bias")
        # consts go via the vector queue: the SP queue must start issuing
        # x-chunk DMAs immediately (the PE waits on those)
        nc.scalar.dma_start(cos_sb[:], cosT[:])
        nc.scalar.dma_start(sin_sb[:], sinT[:])
        nc.scalar.dma_start(tri_sb[:], tri[:])
        nc.scalar.dma_start(id_sb[:], ident[:])
        nc.scalar.dma_start(b_sb[:], bias6[:].rearrange("i p -> p i"))

        # per-batch state
        rot = [{}, {}]   # b -> mg -> rotated [128, S] bf16 (mg: 0=Q0 1=K 3=Q1 4=Q2 5=Q3)
        vsb = [None, None]
        at = [[None] * QH, [None] * QH]
        G = [None, None]

        # ------------- QKV machinery (quantized into closures) -------------
        GRPS = ((0, wA), (1, wB), (2, wC))

        def qkv_quanta(b):
            """Return a list of closures; each emits ~1us of PE work (4 MMs)
            plus its own DMA/ACT/DVE trailing ops. All tile allocation happens
            at closure RUN time so pool WAR deps see already-emitted readers.
            xc DMAs on SP queue, slab DMAs on GPSIMD queue (parallel issue)."""
            Q = []
            xc2 = [None] * 16  # k2 -> [128, 2048] bf16: k-chunks 2k2, 2k2+1
            slabsets = [[None] * 8 for _ in range(3)]
            state = {"v_sb": [], "ps": None}

            def load_xc(k2):
                # full 1024-token rows -> 2KB-per-partition DMA segments
                t = xcpool.tile([128, 2048], BF16, tag="xc", name="xc")
                nc.sync.dma_start(
                    t[:].rearrange("p (c s) -> p c s", c=2),
                    xT[b, ts(k2, 256), :].rearrange("(c p) s -> p c s", c=2),
                )
                xc2[k2] = t

            def load_slab(gi, s):
                t = wpool.tile([128, 1024], BF16, tag="wslab", name="wslab")
                nc.gpsimd.dma_start(
                    t[:].rearrange("p (c m) -> p c m", c=4),
                    GRPS[gi][1][ts(s, 512), :].rearrange(
                        "(c p) m -> p c m", c=4
                    ),
                )
                slabsets[gi][s] = t

            def prefetch():
                for k2 in range(4):
                    load_xc(k2)
                load_slab(0, 0)
                load_slab(0, 1)
            Q.append(prefetch)

            def epilogue(mg, n, b=b):
                ps = state["ps"]
                if mg != 2:
                    if n == 0:
                        rot[b][mg] = kqvpool.tile(
                            [128, S], BF16, tag="kqv", name="rot"
                        )
                    q32 = ropepool.tile([128, 512], F32, tag="q32", name="q32")
                    nc.scalar.activation(
                        q32[:], ps[:], AF.Identity, bias=b_sb[:, mg:mg + 1]
                    )
                    sh = ropepool.tile([128, 512], F32, tag="sh", name="sh")
                    nc.gpsimd.dma_start(sh[0:64, :], q32[64:128, :])
                    nc.gpsimd.dma_start(sh[64:128, :], q32[0:64, :])
                    nc.vector.tensor_mul(q32[:], q32[:], cos_sb[:, ts(n, 512)])
                    nc.vector.tensor_mul(sh[:], sh[:], sin_sb[:, ts(n, 512)])
                    nc.vector.tensor_add(
                        rot[b][mg][:, ts(n, 512)], q32[:], sh[:]
                    )
                else:
                    vt = ropepool.tile([128, 512], BF16, tag="vt", name="vt")
                    nc.scalar.activation(
                        vt[:], ps[:], AF.Identity, bias=b_sb[:, 2:3]
                    )
                    state["v_sb"].append(vt)

            # single sweep over x per grp: 4 psum quarters (m,n) accumulate
            # together, so each xc tile is consumed once per grp at ~4.2us
            # per 1MB — well under HBM delivery rate (no DMA-paced stalls).
            # Quarters borrow the idle attention "sp" psum ring (2 qa + 2 sp).
            for gi in range(3):
                for kg in range(8):  # 8 quanta of 16 MMs each
                    def quantum(kg=kg, gi=gi):
                        if kg == 0:
                            state["qs"] = {
                                (0, 0): psqa.tile(
                                    [128, 512], F32, tag="qa", name="qkvps"
                                ),
                                (0, 1): psqa.tile(
                                    [128, 512], F32, tag="qa", name="qkvps"
                                ),
                                (1, 0): pssp.tile(
                                    [128, 512], F32, tag="sp", name="qkvps"
                                ),
                                (1, 1): pssp.tile(
                                    [128, 512], F32, tag="sp", name="qkvps"
                                ),
                            }
                        qs = state["qs"]
                        # quarter-grouped MM order: quarter i's first MM lands
                        # ~1.05*i us into the quantum, past the point where the
                        # previous grp's epilogue act i has freed the ring slot
                        for m in range(2):
                            for n in range(2):
                                for c4 in range(4):
                                    k = kg * 4 + c4
                                    kq, cc = k // 4, k % 4
                                    nc.tensor.matmul(
                                        qs[(m, n)][:],
                                        slabsets[gi][kq][
                                            :, cc * 256 + m * 128:
                                            cc * 256 + m * 128 + 128
                                        ],
                                        xc2[k // 2][
                                            :, (k % 2) * 1024 + n * 512:
                                            (k % 2) * 1024 + n * 512 + 512
                                        ],
                                        start=(k == 0),
                                        stop=(k == 31),
                                    )
                        # staging (after the MMs so WAR deps resolve fast)
                        if kg < 6:
                            load_slab(gi, kg + 2)
                        elif gi < 2:
                            load_slab(gi + 1, kg - 6)
                        if gi == 0 and kg < 6:
                            load_xc(2 * kg + 4)
                            load_xc(2 * kg + 5)
                        if kg == 7:
                            for m in range(2):
                                mg = gi * 2 + m
                                for n in range(2):
                                    state["ps"] = qs[(m, n)]
                                    epilogue(mg, n)
                    Q.append(quantum)

                if gi == 1:
                    # V transpose right after grp B (V = grp B, m=0)
                    def vtrans(half, b=b):
                        if half == 0:
                            vsb[b] = kqvpool.tile(
                                [128, S], BF16, tag="kqv", name="vsb"
                            )
                        for j in range(4):
                            jj = half * 4 + j
                            vt = state["v_sb"][jj // 4]
                            vp = psqa.tile([128, 128], F32, tag="qa", name="vp")
                            nc.tensor.matmul(
                                vp[:], vt[:, ts(jj % 4, 128)], id_sb[:],
                                start=True, stop=True,
                            )
                            nc.vector.tensor_copy(
                                vsb[b][:, ts(jj, 128)], vp[:]
                            )
                    Q.append(lambda: vtrans(0))
                    Q.append(lambda: vtrans(1))
            return Q

        def run_qkv_inline(b):
            for q in qkv_quanta(b):
                q()

        # ------------- attention machinery -------------
        def emit_scores_softmax(b, h, g, pump):
            """Causal mask is folded into the PE stream: the diagonal 128-
            block gets an extra accumulating matmul id^T @ trineg (additive
            -1e5 above the diagonal), so softmax is just exp(+accum_out),
            reciprocal, and the diag-scale tile — a 3-hop chain the depth-2
            pipeline fully hides."""
            Q_t = rot[b][(0, 3, 4, 5)[h]]
            K_t = rot[b][1]
            plist = []
            for j in range(4):
                W = g * 512 + (j + 1) * 128
                qi = 4 * g + j
                P = ppool.tile([128, W], BF16, tag="psb", name="psb")
                sums = sumpool.tile([128, 1], F32, tag="sums", name="sums")
                if W > 512:
                    spA = pssp.tile([128, 512], F32, tag="sp", name="spA")
                    nc.tensor.matmul(
                        spA[:], Q_t[:, ts(qi, 128)], K_t[:, 0:512],
                        start=True, stop=True,
                    )
                    pump(1)
                    WB = W - 512
                    spB = pssp.tile([128, WB], F32, tag="sp", name="spB")
                    nc.tensor.matmul(
                        spB[:], Q_t[:, ts(qi, 128)], K_t[:, 512:W],
                        start=True, stop=False,
                    )
                    nc.tensor.matmul(
                        spB[:, WB - 128:WB], id_sb[:], tri_sb[:],
                        start=False, stop=True,
                    )
                    nc.scalar.activation(P[:, 0:512], spA[:], AF.Exp, scale=SC)
                    nc.vector.reduce_sum(
                        out=sums[:], in_=P[:, 0:512],
                        axis=mybir.AxisListType.X,
                    )
                    pump(1)
                    sums2 = sumpool.tile([128, 1], F32, tag="sums2", name="sums2")
                    nc.scalar.activation(P[:, 512:W], spB[:], AF.Exp, scale=SC)
                    nc.vector.reduce_sum(
                        out=sums2[:], in_=P[:, 512:W],
                        axis=mybir.AxisListType.X,
                    )
                    nc.vector.tensor_add(sums[:], sums[:], sums2[:])
                else:
                    sp = pssp.tile([128, W], F32, tag="sp", name="sp")
                    nc.tensor.matmul(
                        sp[:], Q_t[:, ts(qi, 128)], K_t[:, 0:W],
                        start=True, stop=False,
                    )
                    nc.tensor.matmul(
                        sp[:, W - 128:W], id_sb[:], tri_sb[:],
                        start=False, stop=True,
                    )
                    pump(1)
                    nc.scalar.activation(P[:], sp[:], AF.Exp, scale=SC)
                    nc.vector.reduce_sum(
                        out=sums[:], in_=P[:],
                        axis=mybir.AxisListType.X,
                    )
                recip = sumpool.tile([128, 1], F32, tag="recip", name="recip")
                nc.vector.reciprocal(recip[:], sums[:])
                Dt = dgpool.tile([128, 128], BF16, tag="diag", name="diag")
                nc.vector.tensor_scalar_mul(Dt[:], id_sb[:], recip[:])
                plist.append((P, Dt))
                pump(1)
            return plist

        def emit_pt(g, plist, pump):
            pts = []
            for kc in range(4 * g + 4):
                jst = max(0, kc - 4 * g)
                ptp = pspt.tile([128, 512], F32, tag="ptp", name="ptp")
                for j in range(jst, 4):
                    nc.tensor.matmul(
                        ptp[:, ts(j, 128)],
                        plist[j][0][:, ts(kc, 128)],
                        plist[j][1][:],
                        start=True, stop=True,
                    )
                pt = ptsbpool.tile([128, 512], BF16, tag="ptsb", name="ptsb")
                if kc % 2 == 0:
                    nc.vector.tensor_copy(
                        pt[:, jst * 128:512], ptp[:, jst * 128:512]
                    )
                else:
                    nc.scalar.copy(
                        pt[:, jst * 128:512], ptp[:, jst * 128:512]
                    )
                pts.append((pt, jst))
                if kc % 2 == 1:
                    pump(1)
            return pts

        def emit_ot(b, h, g, pts):
            ot = psot.tile([128, 512], F32, tag="ot", name="ot")
            nkc = 4 * g + 4
            for kc in range(nkc):
                pt, jst = pts[kc]
                nc.tensor.matmul(
                    ot[:, jst * 128:512],
                    vsb[b][:, ts(kc, 128)],
                    pt[:, jst * 128:512],
                    start=(kc == 0), stop=(kc == nkc - 1),
                )
            if at[b][h] is None:
                at[b][h] = atpool.tile([128, S], BF16, tag="at", name="at")
            # DVE copy: keep the saturated ACT queue clear of psum drains
            nc.vector.tensor_copy(at[b][h][:, ts(g, 512)], ot[:])

        a2o_pending = {}  # key -> a2a output dram tile

        def emit_a2a_trigger_full(b):
            """Single AllToAll for all 4 heads of batch b (1MB)."""
            a2i = dpool.tile([NC, 512, 128], BF16, tag="a2iF", name="a2iF")
            for h in range(QH):
                nc.gpsimd.dma_start(
                    a2i[:, h * 128:(h + 1) * 128, :].rearrange(
                        "d p t -> p d t"
                    ),
                    at[b][h][:].rearrange("p (d t) -> p d t", d=NC),
                )
            a2o = dpool.tile([NC, 512, 128], BF16, tag="a2oF", name="a2oF")
            nc.gpsimd.collective_compute(
                "AllToAll",
                OP.bypass,
                ins=[a2i[:].opt()],
                outs=[a2o[:].opt()],
                replica_groups=RG,
            )
            a2o_pending[(b, "full")] = a2o

        def emit_a2a_gather_full(b):
            if G[b] is None:
                G[b] = gpool.tile([128, 4096], BF16, tag="g", name="g")
            a2o = a2o_pending.pop((b, "full"))
            nc.gpsimd.dma_start(
                G[b][:].rearrange("p (fc t) -> p fc t", fc=32),
                a2o[:].rearrange("s (fl p) t -> p (s fl) t", p=128),
            )

        def emit_a2a_trigger(b, hbase):
            """AllToAll for heads hbase, hbase+1 of batch b (512KB)."""
            a2i = dpool.tile([NC, 256, 128], BF16, tag="a2i", name="a2i")
            for hl in range(2):
                nc.gpsimd.dma_start(
                    a2i[:, hl * 128:(hl + 1) * 128, :].rearrange(
                        "d p t -> p d t"
                    ),
                    at[b][hbase + hl][:].rearrange("p (d t) -> p d t", d=NC),
                )
            a2o = dpool.tile([NC, 256, 128], BF16, tag="a2o", name="a2o")
            nc.gpsimd.collective_compute(
                "AllToAll",
                OP.bypass,
                ins=[a2i[:].opt()],
                outs=[a2o[:].opt()],
                replica_groups=RG,
            )
            a2o_pending[(b, hbase)] = a2o

        def emit_a2a_gather(b, hbase):
            if G[b] is None:
                G[b] = gpool.tile([128, 4096], BF16, tag="g", name="g")
            a2o = a2o_pending.pop((b, hbase))
            for hl in range(2):
                nc.gpsimd.dma_start(
                    G[b][:].rearrange(
                        "p (s four t) -> p s four t", s=NC, four=QH
                    )[:, :, hbase + hl, :],
                    a2o[:, hl * 128:(hl + 1) * 128, :].rearrange(
                        "s p t -> p s t"
                    ),
                )

        def emit_warmup_collective():
            """Tiny AllToAll to absorb the ~11us first-collective spin-up
            while the PE is busy with QKV(b0)."""
            wi = dpool.tile([NC, 1, 128], BF16, tag="wi", name="wi")
            nc.gpsimd.dma_start(
                wi[:].rearrange("d o t -> o d t")[0],
                cos_sb[0:1, 0:NC * 128].rearrange("o (d t) -> o d t", d=NC)[0],
            )
            wo_ = dpool.tile([NC, 1, 128], BF16, tag="wu", name="wu")
            nc.gpsimd.collective_compute(
                "AllToAll",
                OP.bypass,
                ins=[wi[:].opt()],
                outs=[wo_[:].opt()],
                replica_groups=RG,
            )

        def run_attention_interleaved(filler, fill_from=13):
            """Both batches' attention, unit-interleaved 2:1 (b0-heavy early)
            so b0's AllToAll fires at ~62% of the phase. Fillers (o_proj b0
            quanta) are pumped only from unit `fill_from` on — after b0's
            gather has landed."""
            fill = {"q": list(filler), "i": 0}

            def pump(n):
                if fill["i"] < fill_from:
                    return
                for _ in range(n):
                    if fill["q"]:
                        f = fill["q"].pop(0)
                        if f is not None:
                            f()

            order = [
                (b, h, g)
                for b in range(B)
                for h in range(QH)
                for g in range(2)
            ]  # b0 fully first: its AllToAll fires at 50% of the phase
            plists = {
                0: emit_scores_softmax(*order[0], pump),
                1: emit_scores_softmax(*order[1], pump),
            }
            for i in range(len(order)):
                fill["i"] = i
                if i + 2 < len(order):
                    plists[i + 2] = emit_scores_softmax(*order[i + 2], pump)
                b, h, g = order[i]
                pts = emit_pt(g, plists.pop(i), pump)
                emit_ot(b, h, g, pts)
                pump(1)
                if i == 7:
                    # all of b0's heads done: single 1MB AllToAll + gather
                    emit_a2a_trigger_full(0)
                    emit_a2a_gather_full(0)
                elif i == 11:
                    emit_a2a_trigger(1, 0)
                    emit_a2a_gather(1, 0)
                elif i == 15:
                    emit_a2a_trigger(1, 2)
                    emit_a2a_gather(1, 2)
            return fill["q"]  # leftovers

        # ------------- o_proj machinery -------------
        def load_wo_pair(fcp, dmq):
            t = wopool.tile([128, 2048], BF16, tag="wo", name="wopair")
            nc.sync.dma_start(
                t[:].rearrange("p (c q) -> p c q", c=2),
                wo[ts(fcp, 256), ts(dmq, 1024)].rearrange(
                    "(c p) q -> p c q", c=2
                ),
            )
            return t

        def oproj_single_quanta(b, dmq, split_heads=False):
            """o_proj for one batch, one dmq chunk: 2 psum quarters held
            across 16 quanta. With split_heads, even fcp pairs (head-local
            features 0-1 of every core, available after the h01 AllToAll)
            run before odd pairs (h23)."""
            Q = []
            yps = {}
            pairs = {}
            fcp_order = (
                list(range(0, 16, 2)) + list(range(1, 16, 2))
                if split_heads else list(range(16))
            )

            def start():
                yps[0] = psqa.tile([128, 512], F32, tag="qa", name="yp0")
                yps[1] = psqa.tile([128, 512], F32, tag="qa", name="yp1")
                pairs[fcp_order[0]] = load_wo_pair(fcp_order[0], dmq)
                pairs[fcp_order[1]] = load_wo_pair(fcp_order[1], dmq)
            Q.append(start)

            for idx in range(16):
                def quantum(idx=idx):
                    if idx + 2 < 16:
                        nxt = fcp_order[idx + 2]
                        pairs[nxt] = load_wo_pair(nxt, dmq)
                    fcp = fcp_order[idx]
                    pair = pairs.pop(fcp)
                    for c in range(2):
                        fc = 2 * fcp + c
                        for n in range(2):
                            nc.tensor.matmul(
                                yps[n][:],
                                G[b][:, ts(fc, 128)],
                                pair[:, c * 1024 + n * 512:
                                     c * 1024 + (n + 1) * 512],
                                start=(idx == 0 and c == 0),
                                stop=(idx == 15 and c == 1),
                            )
                Q.append(quantum)

            def finish():
                for n in range(2):
                    ys = ypool.tile([128, 512], F32, tag="ysb", name="ys")
                    if n == 0:
                        nc.scalar.copy(ys[:], yps[n][:])
                    else:
                        nc.vector.tensor_copy(ys[:], yps[n][:])
                    nc.gpsimd.dma_start(
                        out[b, :, dmq * 1024 + n * 512:
                            dmq * 1024 + (n + 1) * 512],
                        ys[:],
                    )
            Q.append(finish)
            return Q

        def oproj_joint(dmq):
            """o_proj for both batches on one dmq chunk, fcp-interleaved so
            each wo pair tile is consumed immediately. 4 psum quarters:
            2 from qa tag, 2 from sp tag (attention is done by now)."""
            yps = {}
            yps[(0, 0)] = psqa.tile([128, 512], F32, tag="qa", name="yp00")
            yps[(0, 1)] = psqa.tile([128, 512], F32, tag="qa", name="yp01")
            yps[(1, 0)] = pssp.tile([128, 512], F32, tag="sp", name="yp10")
            yps[(1, 1)] = pssp.tile([128, 512], F32, tag="sp", name="yp11")
            pairs = {0: load_wo_pair(0, dmq), 1: load_wo_pair(1, dmq)}
            for fcp in range(16):
                if fcp + 2 < 16:
                    pairs[fcp + 2] = load_wo_pair(fcp + 2, dmq)
                pair = pairs.pop(fcp)
                for b in range(B):
                    for c in range(2):
                        fc = 2 * fcp + c
                        for n in range(2):
                            nc.tensor.matmul(
                                yps[(b, n)][:],
                                G[b][:, ts(fc, 128)],
                                pair[:, c * 1024 + n * 512:
                                     c * 1024 + (n + 1) * 512],
                                start=(fc == 0), stop=(fc == 31),
                            )
            # drain: alternate copy engines so the 4 copies pipeline
            for b in range(B):
                for n in range(2):
                    ys = ypool.tile([128, 512], F32, tag="ysb", name="ys")
                    if (b + n) % 2 == 0:
                        nc.scalar.copy(ys[:], yps[(b, n)][:])
                    else:
                        nc.vector.tensor_copy(ys[:], yps[(b, n)][:])
                    nc.gpsimd.dma_start(
                        out[b, :, dmq * 1024 + n * 512:
                            dmq * 1024 + (n + 1) * 512],
                        ys[:],
                    )

        # ================= schedule =================
        # Phase 1: QKV(b0) inline (dense PE stream, DMA-paced ramp)
        run_qkv_inline(0)
        # absorb the ~11us first-collective spin-up under QKV(b1)
        emit_warmup_collective()
        # Phase 2: QKV(b1) inline
        run_qkv_inline(1)
        # Phase 3: attention, b0's units first. o_proj(b0, dmq0) quanta pump
        # only into the last two units — by then b0's AllToAll (fired at 50%)
        # and its gather are long done, so pumped MMs never block the queue.
        op0 = oproj_single_quanta(0, 0, split_heads=True)
        leftovers = run_attention_interleaved(op0, fill_from=14)
        for q in leftovers:
            if q is not None:
                q()
        # Phase 4: o_proj(b1, dmq0), h01 pairs first (their AllToAll lands
        # before the h23 one); wo dmq0 re-streamed
        for q in oproj_single_quanta(1, 0, split_heads=True):
            q()
        # Phase 5: joint o_proj dmq 1..3
        for dmq in range(1, 4):
            oproj_joint(dmq)

    if split_for_walrus:
        _split_waits(nc, cap=1)
    return nc


def _split_waits(nc, cap=1):
    """This walrus build accepts at most one sync wait per instruction; hoist
    the excess onto same-engine NoOps inserted immediately before."""
    for fn in nc.m.functions:
        for bb in fn.blocks:
            new_insts = []
            for inst in bb.instructions:
                si = inst.sync_info
                if si is not None and si.on_wait and len(si.on_wait) > cap:
                    waits = list(si.on_wait)
                    head, rest = waits[: len(waits) - cap], waits[len(waits) - cap:]
                    for i in range(0, len(head), cap):
                        nop = mybir.InstNoOp(
                            name=f"{inst.name}-wsplit{i}", ins=[], outs=[]
                        )
                        nop.engine = inst.engine
                        nop.sync_info = mybir.SyncInfo(
                            on_wait=head[i : i + cap], on_update=[]
                        )
                        new_insts.append(nop)
                    inst.sync_info = mybir.SyncInfo(
                        on_wait=rest, on_update=list(si.on_update)
                    )
                new_insts.append(inst)
            bb.instructions = new_insts
    return nc


_NC_CACHE = None


def _get_nc():
    global _NC_CACHE
    if _NC_CACHE is None:
        _NC_CACHE = _build()
    return _NC_CACHE


def _prep_inputs(x, storage_idx, Wq, bq, Wk, bk, Wv, bv, Wo):
    bf = ml_dtypes.bfloat16
    xT = np.ascontiguousarray(
        np.asarray(x, np.float32).transpose(0, 2, 1)
    ).astype(bf)  # [B, D, S]
    wo_bf = np.ascontiguousarray(np.asarray(Wo, np.float32)).astype(bf)

    pos = np.asarray(storage_idx, np.int64).astype(np.float32)  # [S]
    inv = (1.0 / (THETA ** (np.arange(0, HD, 2, dtype=np.float32) / HD))).astype(
        np.float32
    )
    fr = pos[:, None] * inv[None, :]  # [S, 64]
    emb = np.concatenate([fr, fr], axis=1)  # [S, HD]
    cosT = np.ascontiguousarray(np.cos(emb).T.astype(np.float32)).astype(bf)
    sinT32 = np.ascontiguousarray(np.sin(emb).T).astype(np.float32)
    sinT32[0:64] *= -1.0
    sinT = sinT32.astype(bf)  # fold rotate_half sign

    # additive causal mask for diagonal 128-blocks (0 allowed, -1e5 masked);
    # accumulated onto the scores in-PE via id^T @ tri
    r = np.arange(128)[:, None]
    c = np.arange(128)[None, :]
    tri = np.where(c <= r, 0.0, -1e5).astype(np.float32).astype(bf)
    identity = np.eye(128, dtype=np.float32).astype(bf)

    in_maps = []
    for core in range(NC):
        q0 = core * 512
        kv = slice(core * 128, (core + 1) * 128)
        wA = np.ascontiguousarray(
            np.concatenate([Wq[:, q0 : q0 + 128], Wk[:, kv]], axis=1)
        ).astype(bf)
        wB = np.ascontiguousarray(
            np.concatenate([Wv[:, kv], Wq[:, q0 + 128 : q0 + 256]], axis=1)
        ).astype(bf)
        wC = np.ascontiguousarray(Wq[:, q0 + 256 : q0 + 512]).astype(bf)
        bias6 = np.stack(
            [
                np.asarray(bq[q0 : q0 + 128], np.float32),
                np.asarray(bk[core * 128 : (core + 1) * 128], np.float32),
                np.asarray(bv[core * 128 : (core + 1) * 128], np.float32),
                np.asarray(bq[q0 + 128 : q0 + 256], np.float32),
                np.asarray(bq[q0 + 256 : q0 + 384], np.float32),
                np.asarray(bq[q0 + 384 : q0 + 512], np.float32),
            ]
        )  # [6, 128]
        in_maps.append(
            {
                "xT": xT,
                "wA": wA,
                "wB": wB,
                "wC": wC,
                "wo": wo_bf,
                "bias6": np.ascontiguousarray(bias6),
                "cosT": cosT,
                "sinT": sinT,
                "tri": tri,
                "ident": identity,
            }
        )
    return in_maps


_LAST_RESULTS = None


def kernel(x, storage_idx, cache, mask, Wq, bq, Wk, bk, Wv, bv, Wo):
    """Full-input, full-output entry point. cache/mask are consumed implicitly:
    cache is zeros and positions >= S are causally masked, so the computation
    reduces to causal attention over the S prefill tokens."""
    global _LAST_RESULTS
    in_maps = _prep_inputs(x, storage_idx, Wq, bq, Wk, bk, Wv, bv, Wo)
    nc = _get_nc()
    res = run_bass_kernel_spmd(nc, in_maps, core_ids=list(range(NC)))
    _LAST_RESULTS = res
    full = np.empty((B, S, D), np.float32)
    for c in range(NC):
        o = res.results[c]["out"]  # [B, 128, D]
        for b in range(B):
            full[b, 128 * c : 128 * (c + 1), :] = o[b]
    return full
